# revision 1
# baseline (speedup 1.0000x reference)
"""Trainium2 kernel for nn_BalancedHamiltonLayer.

Math: out = einsum("btd,rde->bte", x, factors)/sqrt(rank) + bias.
The einsum contracts r as a plain sum, so sum_r (x @ F_r) == x @ (sum_r F_r):
one [16384,2048] @ [2048,2048] GEMM instead of eight.

Distribution over 8 NeuronCores (single SPMD program):
  - x is row-sharded over b*t: core c owns rows [c*2048, (c+1)*2048).
  - factors are column-sharded for the reduction: core c reduces
    W_c = sum_r factors[r, :, c*256:(c+1)*256] on-device (DVE tree adds),
    then two AllGathers (one per 128-wide e-half of W_c) replicate the
    full W to every core. The e-halves are independent output columns,
    so the GEMM runs e-tile by e-tile: the first AllGather unblocks half
    the GEMM and the second hides completely under it.
  - GEMM per core: x^T stays fully SBUF-resident in fp32r; W e-tiles
    [128d x 128e] stream through SBUF as the stationary operand
    (out^T = W_tile.T @ x^T at full fp32r PE rate). The 1/sqrt(8) scale
    and bias land in the PSUM eviction (DVE tensor_scalar, bias is
    per-partition in the transposed layout).
  - Each core writes out^T [2048e, 2048m]; the host transposes back.

Host side only shards/lays out inputs (partition-major swizzle so every DMA
is contiguous per partition) and reassembles the per-core outputs.
"""

import math

import numpy as np

B, T, DIM, RANK = 4, 4096, 2048, 8
N_CORES = 8
MC = (B * T) // N_CORES        # 2048 rows per core
EC = DIM // N_CORES            # 256 output cols reduced per core
NT = DIM // 128                # 16 contraction tiles
NJ = MC // 256                 # 8 m-super-tiles per core
NET = 2 * N_CORES              # 16 e-tiles of 128 cols
SCALE = 1.0 / math.sqrt(RANK)

_CACHE = {}


def _build():
    import concourse.bacc as bacc
    import concourse.mybir as mybir
    import concourse.tile as tile

    f32 = mybir.dt.float32
    f32r = mybir.dt.float32r
    add = mybir.AluOpType.add
    mult = mybir.AluOpType.mult
    grp = [list(range(N_CORES))]

    nc = bacc.Bacc(
        "TRN2", target_bir_lowering=False, debug=False, num_devices=N_CORES
    )
    # [J, p, t, m]: x^T tiles, d = t*128+p, m_global = J*512+m
    xh = nc.dram_tensor("xh", [NJ // 2, 128, NT, 512], f32r, kind="ExternalInput").ap()
    # [r, eh, p, t, e]: this core's factor slice, d = t*128+p,
    # e_global = 256*core + 128*eh + e
    fh = nc.dram_tensor(
        "fh", [2, RANK // 2, 128, 2, NT, 128], f32, kind="ExternalInput"
    ).ap()
    # [p, et]: bias for e-tile et=(eh*8+r) at partition p:
    # bias_cols[p, et] = bias[r*256 + eh*128 + p]
    bias_cols = nc.dram_tensor("bias_cols", [128, NET], f32, kind="ExternalInput").ap()
    # transposed output: outT[e, m]
    outT = nc.dram_tensor("outT", [DIM, MC], f32, kind="ExternalOutput").ap()

    with tile.TileContext(nc) as tc:
        with (
            tc.tile_pool(name="const", bufs=1) as const_pool,
            tc.tile_pool(name="dram", bufs=1, space="DRAM") as dram_pool,
            tc.tile_pool(name="xa", bufs=1) as xa_pool,
        ):
            scope = nc.named_scope
            bias_sb = const_pool.tile([128, NET], f32)
            nc.sync.dma_start(bias_sb[:], bias_cols[:])

            wc_half = [
                dram_pool.tile([128, NT, 128], f32r, name=f"wc_half{i}")
                for i in range(2)
            ]
            w_half = [
                dram_pool.tile(
                    [N_CORES, 128, NT, 128], f32r,
                    addr_space="Shared", name=f"w_half{i}",
                )
                for i in range(2)
            ]

            # Phase 1: W_c = sum_r fh[r]. One 1 MB load per (rank, e-half),
            # all on the scalar HWDGE ring (sync ring only carries the wc
            # stores, so no head-of-line blocking). DVE tree adds per
            # (e-half, t-half), two leaf adds on GpSimd. Each e-half's
            # AllGather fires as soon as that half is stored.
            with (
                tc.tile_pool(name="red", bufs=6) as red_pool,
                tc.tile_pool(name="racc", bufs=1) as acc_pool,
            ):
                last_fload = None
                for eh in range(2):
                  with scope(f"reduce{eh}"):
                    pr = []
                    for rp in range(RANK // 2):
                        p_t = red_pool.tile([128, 2, NT, 128], f32, tag="fr")
                        eng = nc.scalar if rp % 2 == 0 else nc.sync
                        last_fload = eng.dma_start(p_t[:], fh[eh, rp])  # [p,q,t,e]
                        pr.append(p_t)
                    sA = acc_pool.tile([128, NT, 128], f32, tag="s0")
                    sB = acc_pool.tile([128, NT, 128], f32, tag="s1")
                    nc.vector.tensor_add(sA[:], pr[0][:, 0], pr[0][:, 1])
                    nc.vector.tensor_add(sB[:], pr[1][:, 0], pr[1][:, 1])
                    nc.vector.tensor_add(sA[:], sA[:], sB[:])
                    # sB freed; reused below for the gpsimd half
                    sC = acc_pool.tile([128, NT, 128], f32, tag="s2")
                    nc.vector.tensor_add(sB[:], pr[2][:, 0], pr[2][:, 1])
                    nc.vector.tensor_add(sC[:], pr[3][:, 0], pr[3][:, 1])
                    nc.vector.tensor_add(sB[:], sB[:], sC[:])
                    sfin = acc_pool.tile([128, NT, 128], f32r, tag="sf")
                    nc.vector.tensor_add(sfin[:], sA[:], sB[:])
                    nc.gpsimd.dma_start(wc_half[eh][:], sfin[:])
                    with scope(f"ag{eh}"):
                        nc.gpsimd.collective_compute(
                            "AllGather", mybir.AluOpType.bypass,
                            ins=[wc_half[eh].opt()],
                            outs=[w_half[eh].opt()],
                            replica_groups=grp,
                        )

            # x^T first half on the HWDGE rings: FIFO order guarantees the
            # factor loads drain first. xh is declared f32r (host feeds raw
            # fp32 bits; the PE's f32r path truncates identically).
            xa = xa_pool.tile([128, NT, 2, 512], f32r)
            with tc.tile_wait_until(0.025):
                for J in range(2):
                    eng = nc.sync if J % 2 == 0 else nc.scalar
                    eng.dma_start(xa[:, :, J, :], xh[J])

            # Phase 3: out^T[e,:] per 128-wide e-tile; W tile is the
            # stationary operand, resident x^T streams through the PE.
            with (
                tc.tile_pool(name="xb", bufs=1) as xb_pool,
                tc.tile_pool(name="wsb", bufs=4) as wpool,
                tc.tile_pool(name="osb", bufs=2) as opool,
                tc.tile_pool(name="ps", bufs=2, space="PSUM") as ppool,
            ):
                xb = xb_pool.tile([128, NT, 2, 512], f32r)
                with tc.tile_wait_until(0.045):
                    for J in range(2):
                        eng = nc.sync if J % 2 == 0 else nc.scalar
                        eng.dma_start(xb[:, :, J, :], xh[2 + J])
                for et in range(NET):
                    eh, r = et // N_CORES, et % N_CORES
                    wsb = wpool.tile([128, NT, 128], f32r, tag="wsb")
                    nc.gpsimd.dma_start(wsb[:], w_half[eh][r])
                    with scope(f"gemm{et}"):
                        ps = ppool.tile([128, 4, 512], f32, tag="ps")
                        for mc in range(4):
                            xsrc = xa if mc < 2 else xb
                            for t in range(NT):
                                nc.tensor.matmul(
                                    ps[:, mc, :],
                                    wsb[:, t, :],
                                    xsrc[:, t, mc % 2, :],
                                    start=(t == 0),
                                    stop=(t == NT - 1),
                                )
                        osb = opool.tile([128, MC], f32, tag="osb")
                        nc.vector.tensor_scalar(
                            osb[:], ps.rearrange("p a b -> p (a b)"),
                            SCALE, bias_sb[:, et : et + 1], mult, add,
                        )
                        e0 = r * EC + eh * 128
                        nc.scalar.dma_start(outT[e0 : e0 + 128, :], osb[:])

    nc.compile()
    return nc


def _get_nc():
    if "nc" not in _CACHE:
        _CACHE["nc"] = _build()
    return _CACHE["nc"]


def _shard(x, factors, bias):
    x_flat = np.ascontiguousarray(x, dtype=np.float32).reshape(B * T, DIM)
    factors = np.ascontiguousarray(factors, dtype=np.float32)
    bias = np.ascontiguousarray(bias, dtype=np.float32)
    # bias_cols[p, eh*8+r] = bias[r*256 + eh*128 + p]
    bias_cols = np.ascontiguousarray(
        bias.reshape(RANK, 2, 128).transpose(2, 1, 0).reshape(128, NET)
    )
    in_maps = []
    for c in range(N_CORES):
        xc = x_flat[c * MC : (c + 1) * MC, :]          # [m, d]
        # -> [J, p, t, m_local] with d = t*128+p, m = J*256+m_local
        xh = np.ascontiguousarray(
            xc.T.reshape(NT, 128, NJ // 2, 512).transpose(2, 1, 0, 3)
        )
        fc = factors[:, :, c * EC : (c + 1) * EC]       # [r, d, e]
        # -> [eh, rpair, p, q, t, e128] with r = 2*rpair + q, d = t*128 + p
        fhc = np.ascontiguousarray(
            fc.reshape(RANK // 2, 2, NT, 128, 2, 128).transpose(4, 0, 3, 1, 2, 5)
        )
        in_maps.append({"xh": xh, "fh": fhc, "bias_cols": bias_cols})
    return in_maps


def _run(in_maps, trace=False, trace_cores=None):
    from concourse.bass_utils import run_bass_kernel_spmd

    nc = _get_nc()
    return run_bass_kernel_spmd(
        nc, in_maps, list(range(N_CORES)), trace=trace, trace_cores=trace_cores
    )


def _assemble(res):
    out = np.empty((B * T, DIM), dtype=np.float32)
    for c in range(N_CORES):
        out[c * MC : (c + 1) * MC, :] = res.results[c]["outT"].T
    return out.reshape(B, T, DIM)


def kernel(x, factors, bias):
    res = _run(_shard(x, factors, bias), trace=False)
    return _assemble(res)



# revision 3
# speedup vs baseline: 1.5669x; 1.5669x over previous
"""Trainium2 kernel for nn_BalancedHamiltonLayer.

Math: out = einsum("btd,rde->bte", x, factors)/sqrt(rank) + bias.
The einsum contracts r as a plain sum, so sum_r (x @ F_r) == x @ (sum_r F_r):
one [16384,2048] @ [2048,2048] GEMM instead of eight.

Distribution over 8 NeuronCores (single SPMD program, NO collectives):
tensor-parallel on the output dim. Core c owns output columns
e in [c*256, (c+1)*256):
  - factors are column-sharded: core c loads fh = factors[:, :, c-slice]
    (bf16, 1 MB per rank) and reduces W_c = sum_r fh[r] with a DVE tree —
    W_c [2048d, 256e] stays SBUF-resident for the whole GEMM.
  - x is replicated (host ships the full x^T in bf16 to every core) and
    STREAMED: 32 m-chunks of [2048d, 512m] (2.1 MB each) double-buffered
    through SBUF.
  - GEMM per core: out^T[e, m] = W_c^T @ x^T, 1024 matmuls of N=512
    (stationary = W d-tile [128d,128e], moving = x^T [128d,512m]).
    The 1/sqrt(8) scale and bias land in the PSUM eviction
    (DVE tensor_scalar; bias is per-partition in the transposed layout).
  - Each core writes out^T [256e, 16384m] bf16; the host transposes back.

No AllGather / barrier: cores are fully independent, so launch skew and
collective latency never gate the PE. Everything is bf16 (tolerance is
2e-2; bf16 operand quantization contributes ~1e-2 worst-case).
"""

import math

import ml_dtypes
import numpy as np

B, T, DIM, RANK = 4, 4096, 2048, 8
N_CORES = 8
M = B * T                      # 16384 rows total (replicated to all cores)
EC = DIM // N_CORES            # 256 output cols per core
NT = DIM // 128                # 16 contraction tiles
NJ = M // 512                  # 32 m-chunks of 512
SCALE = 1.0 / math.sqrt(RANK)

_CACHE = {}


def _build():
    import concourse.bacc as bacc
    import concourse.mybir as mybir
    import concourse.tile as tile

    f32 = mybir.dt.float32
    bf16 = mybir.dt.bfloat16
    add = mybir.AluOpType.add
    mult = mybir.AluOpType.mult

    nc = bacc.Bacc(
        "TRN2", target_bir_lowering=False, debug=False, num_devices=N_CORES
    )
    # x^T tiles, replicated: d = t*128+p, m = J*512+j
    xh = nc.dram_tensor("xh", [NJ, 128, NT, 512], bf16, kind="ExternalInput").ap()
    # this core's factor slice: d = t*128+p, e_global = EC*core + e
    fh = nc.dram_tensor("fh", [RANK, 128, NT, EC], bf16, kind="ExternalInput").ap()
    # biasc[p, e2] = bias[EC*core + e2*128 + p]
    biasc = nc.dram_tensor("biasc", [128, 2], f32, kind="ExternalInput").ap()
    # transposed output slice: outT[e_local, m]
    outT = nc.dram_tensor("outT", [EC, M], bf16, kind="ExternalOutput").ap()

    with tile.TileContext(nc) as tc:
        with (
            tc.tile_pool(name="const", bufs=1) as const_pool,
            tc.tile_pool(name="wsb", bufs=1) as wpool,
        ):
            scope = nc.named_scope
            bias_sb = const_pool.tile([128, 2], f32)
            nc.gpsimd.dma_start(bias_sb[:], biasc[:])

            # Phase 1: W_c = sum_r fh[r]. Eight 1 MB loads split across the
            # two HWDGE rings, DVE tree adds (fire as pairs arrive).
            W = wpool.tile([128, NT, EC], bf16)
            with (
                tc.tile_pool(name="red", bufs=8) as red_pool,
                tc.tile_pool(name="racc", bufs=4) as acc_pool,
            ):
                with scope("reduce"):
                    fr = []
                    for r in range(RANK):
                        t_ = red_pool.tile([128, NT, EC], bf16, tag="fr")
                        eng = nc.sync if r % 2 == 0 else nc.scalar
                        eng.dma_start(t_[:], fh[r])
                        fr.append(t_)
                    s = [acc_pool.tile([128, NT, EC], bf16, tag=f"s{i}",
                                       name=f"s{i}")
                         for i in range(4)]
                    for i in range(4):
                        nc.vector.tensor_add(s[i][:], fr[2 * i][:], fr[2 * i + 1][:])
                    nc.vector.tensor_add(s[0][:], s[0][:], s[1][:])
                    nc.vector.tensor_add(s[2][:], s[2][:], s[3][:])
                    nc.vector.tensor_add(W[:], s[0][:], s[2][:])

            # Phase 2: stream x^T chunks, GEMM e-tile by e-tile, evict with
            # scale+bias, store out^T. No cross-core dependencies anywhere.
            with (
                tc.tile_pool(name="xa", bufs=3) as xapool,
                tc.tile_pool(name="osb", bufs=2) as opool,
                tc.tile_pool(name="ps", bufs=3, space="PSUM") as ppool,
            ):
                for J in range(NJ):
                    xa = xapool.tile([128, NT, 512], bf16, tag="xa")
                    eng = nc.sync if J % 2 == 0 else nc.scalar
                    eng.dma_start(xa[:], xh[J])
                    with scope(f"gemm{J}"):
                        ps = ppool.tile([128, 2, 512], f32, tag="ps")
                        for e2 in range(2):
                            for t in range(NT):
                                nc.tensor.matmul(
                                    ps[:, e2, :],
                                    W[:, t, e2 * 128 : (e2 + 1) * 128],
                                    xa[:, t, :],
                                    start=(t == 0),
                                    stop=(t == NT - 1),
                                )
                        osb = opool.tile([128, 2, 512], bf16, tag="osb")
                        for e2 in range(2):
                            nc.vector.tensor_scalar(
                                osb[:, e2, :], ps[:, e2, :],
                                SCALE, bias_sb[:, e2 : e2 + 1], mult, add,
                            )
                        for e2 in range(2):
                            nc.gpsimd.dma_start(
                                outT[e2 * 128 : (e2 + 1) * 128,
                                     J * 512 : (J + 1) * 512],
                                osb[:, e2, :],
                            )

    nc.compile()
    return nc


def _get_nc():
    if "nc" not in _CACHE:
        _CACHE["nc"] = _build()
    return _CACHE["nc"]


def _shard(x, factors, bias):
    bf = ml_dtypes.bfloat16
    x_flat = np.ascontiguousarray(x, dtype=np.float32).reshape(M, DIM)
    factors = np.ascontiguousarray(factors, dtype=np.float32)
    bias = np.ascontiguousarray(bias, dtype=np.float32)
    # xh: [J, p, t, m_local] with d = t*128+p, m = J*512+m_local (replicated)
    xh = np.ascontiguousarray(
        x_flat.T.reshape(NT, 128, NJ, 512).transpose(2, 1, 0, 3).astype(bf)
    )
    in_maps = []
    for c in range(N_CORES):
        fc = factors[:, :, c * EC : (c + 1) * EC]       # [r, d, e]
        fhc = np.ascontiguousarray(
            fc.reshape(RANK, NT, 128, EC).transpose(0, 2, 1, 3).astype(bf)
        )
        biasc = np.ascontiguousarray(
            bias[c * EC : (c + 1) * EC].reshape(2, 128).T
        )
        in_maps.append({"xh": xh, "fh": fhc, "biasc": biasc})
    return in_maps


def _run(in_maps, trace=False, trace_cores=None):
    from concourse.bass_utils import run_bass_kernel_spmd

    nc = _get_nc()
    return run_bass_kernel_spmd(
        nc, in_maps, list(range(N_CORES)), trace=trace, trace_cores=trace_cores
    )


def _assemble(res):
    out = np.empty((M, DIM), dtype=np.float32)
    for c in range(N_CORES):
        out[:, c * EC : (c + 1) * EC] = res.results[c]["outT"].T.astype(np.float32)
    return out.reshape(B, T, DIM)


def kernel(x, factors, bias):
    res = _run(_shard(x, factors, bias), trace=False)
    return _assemble(res)
